# revision 13
# baseline (speedup 1.0000x reference)
"""Bass/Trainium2 kernel for DepthAttentionResidual (bf16 edition).

Math (per (b, t) position, S=16 sources, D=2048):
    ss[s]  = sum_d x[s]^2
    qx[s]  = sum_d q[d] * x[s, d]
    score  = qx * rsqrt(ss/D + eps) / sqrt(D)          # keys never materialized
    w      = softmax_s(score)                          # no max-subtract: |score| ~ N(0,1)
    out[d] = sum_s w[s] * x[s, d]

The rel-err gate is 2e-2; bf16 quantization of x/q/out adds ~2e-3, so the
whole kernel runs on bf16 tensors (fp32 stats/accumulators), halving HBM
traffic vs fp32: 32.5 MB in + 2 MB out per core.

Sharding: data-parallel over (B x T/2) -> 8 cores; each core gets
x_c = source_bank[:, b, half] of shape [16, 512, 2048] (bf16, 32 MB) and
produces [512, 2048] (bf16, upcast to fp32 on host).

On-chip layout: each SBUF x-tile is [128, 2048] with partition p = tl*2 + sq,
covering 64 t-rows (tl) x 2 sources (sq).  A "t-group" of 64 rows spans QT=8
such tiles (source quarters).  Per tile:
  - ACT: Square activation with accum_out  -> ss column   (1.2 G fe/s)
  - DVE: custom TTR (x * q_bcast, accum)   -> qx column   (0.96 G fe/s, 1x)
    (ss slices can be shifted to DVE via SS_DVE if DVE has slack)
  - softmax over s: partition-group sums via a tiny TensorE matmul
    (indicator I32), a free-dim reduce over quarters, reciprocal, and a
    broadcast-back matmul (indicator J4)
  - weighted sum over s: TensorE bf16 matmul with lhsT = w * I32 (sparse
    weights), accumulating over the 8 quarters in PSUM.  The 2 t-groups of a
    batch land at partition offsets 64*g of shared [128, 512] PSUM chunk
    tiles -> 4 full-width PSUM->SBUF copies per batch -> one 512 KB DMA out.

FAST_OPS (off): an attempt at hand-authored 2x_1p custom DVE accumulate ops;
the 2x datapath computes correct elementwise products on HW but the
accumulator drain reads garbage in 2x mode (the accum tap appears to sit
behind the 16-bit write-port packing stage), so the kernel ships with the
stock 1x TTR.
"""

import math

import numpy as np

S, B, T, D = 16, 4, 1024, 2048
N_CORES = 8
SQ = 2                    # sources per partition-group
QT = S // SQ              # 8 source-quarters
TL = 64                   # t-rows per x-tile
G = 2                     # t-groups per batch
BATCH = G * QT            # 16 x-tiles per batch -> 128 t-rows
P = 128
T_CORE = (B * T) // N_CORES          # 512 t-rows per core
EPS = float(np.finfo(np.float32).eps)
SCALE = 1.0 / math.sqrt(D)           # TEMPERATURE = 1.0
SS_DVE = None             # ss slices per batch (of 16) on DVE; None = auto
FAST_OPS = False          # 2x accum tap is broken on HW; keep 1x TTR
FAST_VARIANT = "pairboth"  # 2x program flavor (see _uops2x)
FAST_SCALE = 1.0          # accum_out = FAST_SCALE * true sum for this variant

_module_cache = {}


# --------------------------------------------------------------------------
# Hand-authored 2x_1p custom DVE ops: QX2X (dot-product accumulate) and SQ2X
# (sum-of-squares accumulate), bf16-packed 2 elems/cycle.  The stock
# custom-DVE pipeline only emits the 1x (REGULAR) uop program; the table
# generator and the InstCustomDveAnt byte-36 perf_max field already support
# per-mode programs, so the 2X_1PORT program is authored by hand, mirroring
# the lowered 1x TTR structure: uop0 seeds the accumulator block with
# CONST_0 (BYPASS into its out-flop, one issue-cycle before the first
# product reaches it), uop1 is the steady state (ADD(CURR, PREV)
# self-feedback running total, BYPASS chain to block 7 whose ALU path feeds
# the accum_out drain).  blk0 = lo product, blk1 = hi product, blk2 =
# pair-sum, blk3 = accumulator; delay lanes 5/1 carry the lo/hi elementwise
# products to WR0_LO/WR0_HI.
# --------------------------------------------------------------------------

def _register_fast_ops():
    from operator import add as _add

    import concourse.dve_ops as dmod
    from concourse.dve_spec import Spec, Src0, Src1, C0, lower, sq, _has_src1
    from concourse.dve_uop import (
        UopConfig, DveOpSpec, InpSel, AluInp, DelayInp, AluOp,
        OutPath, OutSel, Trigger,
    )

    EN = 1

    def _mk_uop(lanes, outs, *, trigger0, next0, repeat, req):
        u = UopConfig()
        for li, sel in lanes.items():
            u.enable_input(sel, li)
        u.accum_enabled = EN
        u.repeat_count = repeat
        u.trigger = (trigger0, Trigger.NONE, Trigger.NONE)
        u.next_uop = (next0, 0, 0)
        u.require_inp0, u.require_inp1 = req
        for p in OutPath:
            u.out_enable[p] = 0
        for path, sel in outs.items():
            u.out[path] = sel
            u.out_enable[path] = EN
        return u

    def _dp(u, bi, op, s0, s1, delays, outa=False):
        dp = u.datapath_config[bi]
        dp.op = op
        dp.alu_src0 = s0
        dp.alu_src1 = s1
        dp.alu_out_enable = EN
        dp.alu_out_a_enable = EN if outa else 0
        dp.delay = [DelayInp.PREV_ALU_OUT] * len(dp.delay)
        dp.delay_enable = [0] * len(dp.delay_enable)
        for dl, src in delays.items():
            dp.delay[dl] = src
            dp.delay_enable[dl] = EN

    A, DL = AluInp, DelayInp

    def _uops2x(variant):
        lanes = {1: InpSel.SRC_0, 2: InpSel.SRC_1, 3: InpSel.SRC_0_HI,
                 4: InpSel.SRC_1_HI, 5: InpSel.CONST_0}
        accumulate = variant != "pairboth"
        if variant == "orig":
            steady_outs = {OutPath.WR0_LO: OutSel.DELAY_5,
                           OutPath.WR0_HI: OutSel.DELAY_1}
        else:  # pairboth / runboth: final-stage ALU value on both halves
            steady_outs = {OutPath.WR0_LO: OutSel.ALU_OUT,
                           OutPath.WR0_HI: OutSel.ALU_OUT}
        uops = []
        for seed in (True, False):
            u = _mk_uop(
                lanes,
                outs={} if seed else steady_outs,
                trigger0=Trigger.COUNT if seed else Trigger.SRC_TENSOR_DONE,
                next0=1 if seed else 0,
                repeat=1 if seed else 0,
                req=(0, 0) if seed else (1, 1),
            )
            _dp(u, 0, AluOp.MULTIPLY, A.PREV_DELAY_0, A.PREV_DELAY_1,
                {2: DL.PREV_DELAY, 3: DL.PREV_DELAY, 4: DL.PREV_DELAY})
            _dp(u, 1, AluOp.MULTIPLY, A.PREV_DELAY_2, A.PREV_DELAY_3,
                {4: DL.PREV_DELAY, 5: DL.PREV_ALU_OUT})
            _dp(u, 2, AluOp.ADD, A.PREV_ALU_OUT, A.PREV_DELAY_5,
                {1: DL.PREV_ALU_OUT, 4: DL.PREV_DELAY,
                 5: DL.PREV_DELAY}, outa=True)
            if not accumulate:
                _dp(u, 3, AluOp.BYPASS, A.PREV_ALU_OUT, A.PREV_ALU_OUT,
                    {1: DL.PREV_DELAY, 5: DL.PREV_DELAY}, outa=True)
            elif seed:
                _dp(u, 3, AluOp.BYPASS, A.PREV_DELAY_4, A.PREV_DELAY_4,
                    {1: DL.PREV_DELAY, 5: DL.PREV_DELAY}, outa=True)
            else:
                _dp(u, 3, AluOp.ADD, A.CURR_ALU_OUT, A.PREV_ALU_OUT,
                    {1: DL.PREV_DELAY, 5: DL.PREV_DELAY}, outa=True)
            for bi in range(4, 8):
                _dp(u, bi, AluOp.BYPASS, A.PREV_ALU_OUT, A.PREV_ALU_OUT,
                    {1: DL.PREV_DELAY, 5: DL.PREV_DELAY}, outa=True)
            uops.append(u)
        return uops

    def _ref_qx(in0, in1, c0, c1, c2):
        b = (in0.astype(np.float32) * in1.astype(np.float32)).astype(np.float32)
        return b, c0 + b.reshape(b.shape[0], -1).sum(axis=-1, keepdims=True)

    def _ref_sq(in0, in1, c0, c1, c2):
        b = (in0.astype(np.float32) * in0.astype(np.float32)).astype(np.float32)
        return b, c0 + b.reshape(b.shape[0], -1).sum(axis=-1, keepdims=True)

    class _HandOp:
        """Duck-typed stand-in for dve_ops.DveOp with hand 2x uops."""

        def __init__(self, name, spec, variant):
            self.name = name
            self.spec = spec
            self.subdim = False
            self._variant = variant
            self._cache = {}

        def compile(self, ver):
            if ver not in self._cache:
                self._cache[ver] = DveOpSpec(
                    name=self.name,
                    opcode=dmod.get_dve_sub_opcode(self.name),
                    uops=lower(self.spec, ver=ver),
                    uops_2x=_uops2x(self._variant),
                    rd1_en=_has_src1(self.spec),
                )
            return self._cache[ver]

    qx_op = _HandOp(
        "QXF_ANT",
        Spec(body=Src0 * Src1, accum=_add, accum_init=C0, reference=_ref_qx),
        FAST_VARIANT,
    )
    for op in (qx_op,):
        if op.name not in dmod._SUB_OPCODE_FOR_NAME:
            row = dmod._CUSTOM_DVE_ROW_BASE + len(dmod.OPS)
            dmod.OPS.append(op)
            dmod._SUB_OPCODE_FOR_NAME[op.name] = row
    qx_op = next(o for o in dmod.OPS if o.name == "QXF_ANT")
    return qx_op, qx_op


def build_module(t_core=T_CORE, x_bufs=5, groups_per_batch=G, reps=1,
                 ss_dve=SS_DVE, fast=FAST_OPS):
    import concourse.bass as bass
    import concourse.bacc as bacc
    import concourse.mybir as mybir
    import concourse.tile as tile
    from concourse.dve_ops import TENSOR_TENSOR_REDUCE as TTR_OP

    if ss_dve is None:
        ss_dve = 5 if fast else 0
    if fast:
        QX_OP, SQ_OP = _register_fast_ops()

    fp32 = mybir.dt.float32
    bf16 = mybir.dt.bfloat16
    gn = groups_per_batch
    batch = gn * QT
    rows_per_batch = TL * gn
    n_batches = t_core // rows_per_batch
    assert n_batches * rows_per_batch == t_core

    nc = bacc.Bacc(None)
    x_h = nc.declare_dram_parameter("x", [S, t_core, D], bf16, isOutput=False)
    q_h = nc.declare_dram_parameter("q", [D], bf16, isOutput=False)
    o_h = nc.declare_dram_parameter("out", [t_core, D], bf16, isOutput=True)

    # Indicator matrices for partition-group ops (partition p = tl*4 + sq).
    i32 = np.zeros((P, TL), np.float32)
    i32[np.arange(P), np.arange(P) // SQ] = 1.0        # group-sum over sq
    j4 = np.zeros((TL, P), np.float32)
    j4[np.arange(P) // SQ, np.arange(P)] = 1.0         # broadcast back per group
    i32_h = nc.inline_tensor(i32, name="i32const")
    j4_h = nc.inline_tensor(j4, name="j4const")

    x_ap = x_h[:]
    q_ap = q_h[:]
    q_bcast = bass.AP(tensor=q_ap.tensor, offset=q_ap.offset, ap=[[0, P], *q_ap.ap])

    AF = mybir.ActivationFunctionType
    OP = mybir.AluOpType

    with tile.TileContext(nc) as tc:
        with (
            tc.tile_pool(name="xpool", bufs=x_bufs) as xpool,
            tc.tile_pool(name="single", bufs=1) as single,
            tc.tile_pool(name="stats", bufs=4) as stats,
            tc.tile_pool(name="wepool", bufs=4) as wepool,
            tc.tile_pool(name="opool", bufs=2) as opool,
            tc.tile_pool(name="ppool", bufs=4, space="PSUM") as ppool,
            tc.tile_pool(name="pspool", bufs=1, space="PSUM") as pspool,
            tc.tile_pool(name="warmp", bufs=1, space="PSUM") as warmp,
        ):
            q_sb = single.tile([P, D], bf16)
            nc.sync.dma_start(out=q_sb, in_=q_bcast)
            i32_sb = single.tile([P, TL], fp32)
            nc.sync.dma_start(out=i32_sb, in_=i32_h[:])
            j4_sb = single.tile([TL, P], fp32)
            nc.sync.dma_start(out=j4_sb, in_=j4_h[:])
            # discarded elementwise outputs (only accum_out is consumed)
            g_act = single.tile([P, D], bf16)
            g_dve = single.tile([P, D], bf16)

            import contextlib
            rep_ctx = (
                tc.For_i(0, reps, 1) if reps > 1 else contextlib.nullcontext()
            )
            with rep_ctx:
                for bi in range(n_batches):
                    ss_col = stats.tile([P, batch], fp32, tag="ss")
                    qx_col = stats.tile([P, batch], fp32, tag="qx")
                    ostage = opool.tile([rows_per_batch, D], bf16, tag="os")
                    xt = []
                    for g in range(gn):
                        t0 = bi * rows_per_batch + g * TL
                        # 4 x 512KB DMAs per t-group into one [P, QT, D] tile
                        xs = xpool.tile([P, QT, D], bf16, tag="x")
                        for qt in range(QT):
                            src = x_ap[
                                qt * SQ : (qt + 1) * SQ, t0 : t0 + TL, :
                            ].rearrange("s tl d -> tl s d")
                            nc.sync.dma_start(out=xs[:, qt, :], in_=src)
                        xt.append(xs)
                        for qt in range(QT):
                            j = g * QT + qt
                            if (j % batch) < ss_dve:
                                # DVE also handles this ss slice (balance);
                                # squares via QX2X(x, x): two-src keeps the
                                # engine's mode detect at 2x_1p (a single-src
                                # op would try 2x_2p/4x slots).
                                if fast:
                                    bi_ = nc.vector._custom_dve(
                                        QX_OP, out=g_dve, in0=xs[:, qt, :],
                                        in1=xs[:, qt, :], s0=0.0, s1=0.0,
                                        accum_out=ss_col[:, j : j + 1],
                                    )
                                    bi_.ins.perf_max = 1
                                else:
                                    nc.vector._custom_dve(
                                        TTR_OP, out=g_dve, in0=xs[:, qt, :],
                                        in1=xs[:, qt, :], s0=0.0, s1=1.0,
                                        accum_out=ss_col[:, j : j + 1],
                                    )
                            else:
                                nc.scalar.activation(
                                    out=g_act, in_=xs[:, qt, :], func=AF.Square,
                                    accum_out=ss_col[:, j : j + 1],
                                )
                            # ISA TENSOR_TENSOR_REDUCE crashes at runtime here;
                            # custom-DVE ucode variants work.
                            if fast:
                                bi_ = nc.vector._custom_dve(
                                    QX_OP, out=g_dve, in0=xs[:, qt, :],
                                    in1=q_sb, s0=0.0, s1=0.0,
                                    accum_out=qx_col[:, j : j + 1],
                                )
                                bi_.ins.perf_max = 1
                            else:
                                nc.vector._custom_dve(
                                    TTR_OP, out=g_dve, in0=xs[:, qt, :],
                                    in1=q_sb, s0=0.0, s1=1.0,
                                    accum_out=qx_col[:, j : j + 1],
                                )
                            # 1-row dummy matmul tied to this load keeps the
                            # PE clock-gate (HAM) warm between real bursts
                            wpo = warmp.tile([TL, 1], fp32, tag="wp")
                            nc.tensor.matmul(
                                wpo, i32_sb, ss_col[:, 0:1],
                                start=True, stop=True,
                            )

                    # --- batch softmax over the 16 sources (per t-row) ---
                    # rsqrt(ms+eps) via Newton on DVE: avoids the Sqrt ACT
                    # table, so the only table set loaded is exp_and_friends
                    v = stats.tile([P, batch], fp32, tag="v")
                    nc.vector.tensor_scalar(
                        out=v, in0=ss_col, scalar1=1.0 / D, scalar2=EPS,
                        op0=OP.mult, op1=OP.add,
                    )
                    y = stats.tile([P, batch], fp32, tag="y")
                    nc.vector.tensor_scalar(
                        out=y, in0=v, scalar1=-0.5, scalar2=1.5,
                        op0=OP.mult, op1=OP.add,
                    )
                    for _ in range(2):
                        y2 = stats.tile([P, batch], fp32, tag="y2")
                        nc.vector.tensor_mul(y2, y, y)
                        vy2 = stats.tile([P, batch], fp32, tag="vy2")
                        nc.vector.tensor_mul(vy2, v, y2)
                        h = stats.tile([P, batch], fp32, tag="h")
                        nc.vector.tensor_scalar(
                            out=h, in0=vy2, scalar1=-0.5, scalar2=1.5,
                            op0=OP.mult, op1=OP.add,
                        )
                        yn = stats.tile([P, batch], fp32, tag="yn")
                        nc.vector.tensor_mul(yn, y, h)
                        y = yn

                    sc = stats.tile([P, batch], fp32, tag="sc")
                    nc.vector.tensor_mul(sc, qx_col, y)
                    u = stats.tile([P, batch], fp32, tag="u")
                    nc.scalar.activation(out=u, in_=sc, func=AF.Exp, scale=SCALE)

                    pd = pspool.tile([TL, batch], fp32, tag="pd")
                    nc.tensor.matmul(pd, i32_sb, u, start=True, stop=True)
                    dsum = stats.tile([TL, gn], fp32, tag="dsum")
                    nc.vector.tensor_reduce(
                        out=dsum,
                        in_=pd.rearrange("p (g qt) -> p g qt", qt=QT),
                        axis=mybir.AxisListType.X,
                        op=OP.add,
                    )
                    rd = stats.tile([TL, gn], fp32, tag="rd")
                    nc.vector.reciprocal(out=rd, in_=dsum)
                    pbc = pspool.tile([P, gn], fp32, tag="pbc")
                    nc.tensor.matmul(pbc, j4_sb, rd, start=True, stop=True)
                    bc_sb = stats.tile([P, gn], fp32, tag="bc")
                    nc.vector.tensor_copy(bc_sb, pbc)
                    wn = stats.tile([P, batch], fp32, tag="wn")
                    for g in range(gn):
                        nc.vector.tensor_scalar(
                            out=wn[:, g * QT : (g + 1) * QT],
                            in0=u[:, g * QT : (g + 1) * QT],
                            scalar1=bc_sb[:, g : g + 1],
                            scalar2=None,
                            op0=OP.mult,
                        )

                    # sparse bf16 weight tiles (ACT: out = i32 * wn_j), then
                    # the weighted sum on PE, PSUM-accumulated over quarters.
                    # The 4 t-groups write partition ranges [32g, 32g+32) of
                    # shared [128, 512] chunk tiles.
                    we = []
                    for g in range(gn):
                        we_all = wepool.tile([P, QT, TL], bf16, tag="wea")
                        for qt in range(QT):
                            j = g * QT + qt
                            nc.scalar.activation(
                                out=we_all[:, qt, :], in_=i32_sb,
                                func=AF.Copy, scale=wn[:, j : j + 1],
                            )
                        we.append(we_all)
                    for ci, c0 in enumerate(range(0, D, 512)):
                        po = ppool.tile([P, 512], fp32, tag="po")
                        for g in range(gn):
                            for qt in range(QT):
                                nc.tensor.matmul(
                                    po[g * TL : (g + 1) * TL, :],
                                    we[g][:, qt, :],
                                    xt[g][:, qt, c0 : c0 + 512],
                                    start=(qt == 0), stop=(qt == QT - 1),
                                )
                        # full-width PSUM->SBUF copy, fp32 -> bf16
                        nc.scalar.copy(out=ostage[:, c0 : c0 + 512], in_=po)
                    nc.scalar.dma_start(
                        out=o_h[
                            bi * rows_per_batch : (bi + 1) * rows_per_batch, :
                        ],
                        in_=ostage,
                    )

    nc.compile()
    return nc


def _get_module():
    key = (T_CORE, SS_DVE, FAST_OPS)
    if key not in _module_cache:
        _module_cache[key] = build_module()
    return _module_cache[key]


def _to_bf16(a):
    import ml_dtypes
    return np.ascontiguousarray(np.asarray(a, dtype=ml_dtypes.bfloat16))


def _run(layer_query, source_bank, **spmd_kwargs):
    from concourse.bass_utils import run_bass_kernel_spmd

    q = _to_bf16(layer_query)
    x = np.asarray(source_bank, dtype=np.float32)
    assert x.shape == (S, B, T, D)

    nc = _get_module()
    in_maps = []
    for c in range(N_CORES):
        b, h = c // 2, c % 2
        xc = _to_bf16(x[:, b, h * T_CORE : (h + 1) * T_CORE, :])
        in_maps.append({"x": xc, "q": q})

    res = run_bass_kernel_spmd(nc, in_maps, core_ids=list(range(N_CORES)), **spmd_kwargs)
    full = np.empty((B, T, D), dtype=np.float32)
    for c in range(N_CORES):
        b, h = c // 2, c % 2
        full[b, h * T_CORE : (h + 1) * T_CORE, :] = np.asarray(
            res.results[c]["out"], dtype=np.float32
        )
    return full, res


def kernel(layer_query, source_bank, num_sources=None):
    full, _ = _run(layer_query, source_bank)
    return full
